# revision 61
# baseline (speedup 1.0000x reference)
"""HGTConv Trainium2 kernel (8 NeuronCores, dst-sharded, batched SWDGE gather).

Math: softmax over the H=8 head axis followed by attn.mean(axis=-1) is
identically 1/8, so the attention branch drops out:

    out_dst = relu( segsum_dst(x_src[src]) @ Wbig * r8 + xres' )
    Wbig  = Wv @ Wm @ Wout
    r8    = 1/(8*max(cnt,1))                       (per dst node)
    xres' = x_dst + (cnt*r8)*bbig + bout           (host-folded residual)
    bbig  = (bv @ Wm + bm) @ Wout

Sharding: each core owns 1/8 of user dst nodes and 1/8 of game dst nodes,
and receives exactly the edges pointing into them. No collectives.

Gather: source rows are fetched in fp8 (e4m3) with batched `dma_gather`
(SWDGE, 1024 descriptors per call = the ring limit; ~1.3us ucode gen per
call; a dummy gather absorbs the ~10us one-time ucode cold load). int16
gather indices limit the addressable range to 32768 rows, so each source
table is split into banks and one gather per (split, bank) is issued, in
issue order matching tile-major consumption; idx tables are loaded in
pieces so the first gather unblocks early.

Dst-node-to-tile assignment is bin-packed on the host (LPT) so that each
(tile, bank) group holds at most 128 edges (one 128-slot chunk); a few
"fat" tiles with a 256-edge budget absorb heavy nodes. This gives a fixed
chunk structure shared by all 8 cores (single SPMD program).

Scatter within a tile: one-hot matmul. For each chunk, a [128 slots, 128
dst] fp8 selection matrix M (HOST-precomputed, DMA'd via the scalar HWDGE
queue -- the DVE is_equal build was the consumption bottleneck) turns the
segment-sum into PE matmuls accumulating S^T in PSUM; stage 2 multiplies
by the folded Wbig and adds the residual IN PSUM via an identity-matmul
of the host-folded xres; the r8 scale rides the final relu (scalar
activation / DVE tensor_scalar, alternating per tile). The two sides'
segments are interleaved so DMA-heavy (u) and compute-heavy (g) phases
average out; M/xres are prefetched 3 sched steps ahead.
"""

import bisect
import math
from contextlib import ExitStack

import numpy as np
import ml_dtypes

import concourse.tile as tile
import concourse.mybir as mybir
from concourse import bacc
from concourse.bass_utils import run_bass_kernel_spmd

P = 128
D = 256
BF16 = ml_dtypes.bfloat16
F8 = ml_dtypes.float8_e4m3
# idxs per dma_gather = MAXCH*128; bounded by the SWDGE descriptor ring
# (dynamic_dma_scratch_size/16 descriptors). Fewer, bigger gathers amortize
# the ~1.2us/gather stream overhead (gen/drain serialization).
MAXCH = 8
DMA_SCRATCH = 16384

# full-size problem config; side u: dst=user src=game, side g: dst=game src=user
CFG_FULL = dict(
    ncores=8,
    n_user=100000,
    n_game=50000,
    u=dict(T=99, fat=3, nbanks=2, nsegs=6),
    g=dict(T=50, fat=2, nbanks=4, nsegs=6),
)


class PackError(Exception):
    pass


_DEBUG_BUILD = False  # set True for CoreSim runs (keeps debug info)


# ------------------------------------------------------------ structure

def _side_structure(scfg):
    """Core-independent chunk/event layout for one side."""
    T, fat, B, nsegs = scfg["T"], scfg["fat"], scfg["nbanks"], scfg["nsegs"]
    # fat tiles at the FRONT so the tail segments are light
    nch_tile = [2 if t < fat else 1 for t in range(T)]  # chunks per (t, b)

    # segments: contiguous tile ranges, tail segments small to shorten the
    # compute drain after the last gather completes
    w = [1.0] * (nsegs - 2) + [0.5, 0.25]
    cw = np.cumsum([0.0] + w) / sum(w)
    bounds = sorted({int(round(c * T)) for c in cw} | {0, T})
    segs = [(bounds[i], bounds[i + 1]) for i in range(len(bounds) - 1)]

    # event order (ld columns / matmul order): tile-major
    ev_of = {}
    n_ev = 0
    for t in range(T):
        for b in range(B):
            for j in range(nch_tile[t]):
                ev_of[(t, b, j)] = n_ev
                n_ev += 1

    # gather-position order: bank-major (each bank is one contiguous chunk
    # stream, split into <=MAXCH-chunk gathers that may cross segments)
    gpos_of = {}
    bank_base = []
    pos = 0
    for b in range(B):
        bank_base.append(pos)
        for t in range(T):
            for j in range(nch_tile[t]):
                gpos_of[(t, b, j)] = pos
                pos += 1
    n_pos = pos
    chunks_per_bank = n_pos // B

    # uniform split sizes (shared by all banks); remainder in the last split
    szs = []
    rem = chunks_per_bank
    while rem > 0:
        take = min(MAXCH, rem)
        szs.append(take)
        rem -= take
    scum = [0]
    for s in szs:
        scum.append(scum[-1] + s)

    # gather issue order: split-major, bank-minor (matches tile-major
    # consumption so the sliding gather-tile pool never deadlocks).
    # iofs = chunk offset of this gather's idx block in the issue-ordered
    # idx tensor (idx is packed in issue order so piece loads are prefixes).
    gathers = []  # (b, split_idx, gpos_lo, nch, iofs)
    iofs = 0
    for s in range(len(szs)):
        for b in range(B):
            gathers.append((b, s, bank_base[b] + scum[s], szs[s], iofs))
            iofs += szs[s]

    # idx DMA piece boundaries (in gather-issue index): a tiny first piece so
    # gather #1 unblocks early, then thirds
    npg = len(gathers)
    pieces = [0, 2]
    for k in range(3):
        pieces.append(2 + ((npg - 2) * (k + 1)) // 3)

    return dict(
        T=T, fat=fat, B=B, nch_tile=nch_tile, segs=segs,
        ev_of=ev_of, n_ev=n_ev, gpos_of=gpos_of, n_pos=n_pos,
        bank_base=bank_base, gathers=gathers, szs=szs, scum=scum,
        pieces=pieces,
    )


def _structures(cfg):
    return dict(u=_side_structure(cfg["u"]), g=_side_structure(cfg["g"]))


# ------------------------------------------------------------ host packing

def _lpt_binpack(cnt, T, nch_tile, cap_chunk=128):
    """Assign each dst node to a tile s.t. per-(tile,bank) load <= cap and
    <=128 nodes per tile. cnt: [n_nodes, B] int. Returns tile_of [n_nodes]."""
    n, B = cnt.shape
    if n > T * P:
        raise PackError(f"{n} nodes > {T * P} slots")
    caps = np.broadcast_to(
        (np.array(nch_tile, np.int64) * cap_chunk)[:, None], (T, B)
    ).copy()
    slots = np.full(T, P, np.int64)
    tile_of = np.full(n, -1, np.int64)
    tot = cnt.sum(1)
    order = np.argsort(-cnt.max(1), kind="stable")
    nz = order[tot[order] > 0]
    for node in nz:
        c = cnt[node]
        feas = (slots > 0) & (caps >= c).all(1)
        if not feas.any():
            raise PackError("no feasible tile (escalate fat budget)")
        score = caps.sum(1).astype(np.float64)
        score[~feas] = -1.0
        t = int(np.argmax(score))
        tile_of[node] = t
        caps[t] -= c
        slots[t] -= 1
    z = order[tot[order] == 0]
    zi = 0
    for t in range(T):
        k = int(slots[t])
        if k > 0 and zi < len(z):
            take = z[zi : zi + k]
            tile_of[take] = t
            slots[t] -= len(take)
            zi += len(take)
    if zi < len(z):
        raise PackError("not enough node slots")
    return tile_of


def _pack_side(st, dst_local, src, bank_sz, n_dst_slice, x_dst, Wbig, bbig, bout):
    """Host packing of one core-side. Returns dict of device arrays + unpack maps."""
    T, B, n_ev, n_pos = st["T"], st["B"], st["n_ev"], st["n_pos"]
    nch_tile, ev_of, gpos_of = st["nch_tile"], st["ev_of"], st["gpos_of"]

    bank = (src // bank_sz).astype(np.int64)
    sib = (src - bank * bank_sz).astype(np.int64)  # src row within bank

    cnt = np.zeros((n_dst_slice, B), np.int64)
    np.add.at(cnt, (dst_local, bank), 1)
    tile_of = _lpt_binpack(cnt, T, st["nch_tile"])

    # slot within tile: stable order by node id
    order = np.argsort(tile_of, kind="stable")
    slot_of = np.empty(n_dst_slice, np.int64)
    tile_sorted = tile_of[order]
    first = np.searchsorted(tile_sorted, np.arange(T))
    slot_of[order] = np.arange(n_dst_slice) - first[tile_sorted]
    assert slot_of.max() < P

    # group edges by (tile, bank), sorted by src row for DMA locality
    g = tile_of[dst_local] * B + bank
    eorder = np.lexsort((sib, g))
    gs = g[eorder]
    ds = dst_local[eorder]
    ss = sib[eorder]
    gfirst = np.searchsorted(gs, np.arange(T * B))
    gcount = np.diff(np.searchsorted(gs, np.arange(T * B + 1)))
    within = np.arange(len(gs)) - gfirst[gs]

    t_e = gs // B
    b_e = gs % B
    j_e = within >> 7
    p_e = within & 127
    nch_e = np.array(st["nch_tile"], np.int64)[t_e]
    if (j_e >= nch_e).any():
        raise PackError("chunk budget overflow")

    gpos_tab = np.zeros((T, B, 2), np.int64)
    ev_tab = np.zeros((T, B, 2), np.int64)
    for (t, b, j), v in gpos_of.items():
        gpos_tab[t, b, j] = v
    for (t, b, j), v in ev_of.items():
        ev_tab[t, b, j] = v
    gpos_e = gpos_tab[t_e, b_e, j_e]
    ev_e = ev_tab[t_e, b_e, j_e]

    # gather indices (global gather position i -> src row in bank), pad = 0
    idx_flat = np.zeros(n_pos * P, np.int64)
    idx_flat[gpos_e * P + p_e] = ss
    # reorder into gather-issue order (each gather's chunk block contiguous)
    idx_issue = np.concatenate(
        [idx_flat[lo * P : (lo + nch) * P] for (_b, _s, lo, nch, _i) in st["gathers"]]
    )
    assert idx_issue.size == n_pos * P
    # int16 layout [128, n_pos*8]: value i at [i%16, i//16], replicated x8
    idx16 = np.zeros((P, n_pos * 8), np.int16)
    block = idx_issue.reshape(-1, 16).T.astype(np.int16)  # [16, n_pos*8]
    for k in range(8):
        idx16[k * 16 : (k + 1) * 16] = block

    # ld: local dst slot per chunk slot, -1 for dummies
    ld = np.full((P, n_ev), -1.0, np.float32)
    ld[p_e, ev_e] = slot_of[ds].astype(np.float32)
    # one-hot selection matrices, host-built: M[p, ev*128+j] = (ld[p,ev]==j)
    mfull = (ld[:, :, None] == np.arange(P, dtype=np.float32)[None, None, :])
    mfull = np.ascontiguousarray(mfull.reshape(P, n_ev * P).astype(F8))

    # per-node scales
    ctot = cnt.sum(1).astype(np.float32)
    r8 = 1.0 / (8.0 * np.maximum(ctot, 1.0))

    # per-(tile,slot) r8 in pm layout [slot, tile]; holes -> 1/8
    r8pm = np.full((P, T), 1.0 / 8.0, np.float32)
    r8pm[slot_of, tile_of] = r8
    r8pm = r8pm / 16.0  # compensates the x16 in the fp8 DoubleRow weights

    # folded residual, PRE-DIVIDED by r8 (the r8 scale is applied once at the
    # final relu), pm layout [slot, tile*D]
    xres_n = (x_dst + (ctot * r8)[:, None] * bbig[None, :] + bout[None, :]) / (r8[:, None] / 16.0)
    xres = np.zeros((P, T * D), np.float32)
    xres[slot_of[:, None], (tile_of * D)[:, None] + np.arange(D)[None, :]] = xres_n

    return dict(
        idx=idx16,
        ld=np.ascontiguousarray(ld.astype(np.int8)),
        mfull=mfull,
        r8pm=np.ascontiguousarray(r8pm),
        xres=np.ascontiguousarray(xres.astype(BF16)),
        tile_of=tile_of,
        slot_of=slot_of,
    )


def _fold_weights(Wv, bv, Wm, bm, Wout, bout):
    Wbig = (np.float32(Wv) @ np.float32(Wm)) @ np.float32(Wout)
    bbig = (np.float32(bv) @ np.float32(Wm) + np.float32(bm)) @ np.float32(Wout)
    # fp8 DoubleRow layout [p, i, fo] = 16*Wbig[i*128+p, fo]: one double-
    # pumped matmul contracts all 256 input features; the x16 keeps the
    # small (sigma~1/16) weights in e4m3 normal range and is compensated in
    # the r8 relu scale
    wdr = (16.0 * Wbig.reshape(2, P, D).transpose(1, 0, 2)).reshape(P, 2 * D)
    return np.ascontiguousarray(wdr).astype(F8), bbig, np.float32(bout)


# ------------------------------------------------------------ device build

def _build(cfg, sts):
    f32 = mybir.dt.float32
    bf = mybir.dt.bfloat16
    f8 = mybir.dt.float8e4
    i16 = mybir.dt.int16
    i8 = mybir.dt.int8

    nc = bacc.Bacc(
        "TRN2",
        target_bir_lowering=False,
        debug=_DEBUG_BUILD,
        num_devices=cfg["ncores"],
        num_swdge_queues=4,
        dynamic_dma_scratch_size=DMA_SCRATCH,
    )

    bank_cfg = dict(
        u=("xg", cfg["n_game"]),   # side u gathers from game banks
        g=("xu", cfg["n_user"]),   # side g gathers from user banks
    )
    sides = []
    for name in ("u", "g"):
        st = sts[name]
        pre, n_src = bank_cfg[name]
        B = st["B"]
        bank_sz = n_src // B
        side = dict(name=name, st=st, bank_sz=bank_sz)
        side["banks"] = [
            nc.dram_tensor(f"{pre}{b}", [bank_sz, D], f8, kind="ExternalInput")
            for b in range(B)
        ]
        T = st["T"]
        side["idx"] = nc.dram_tensor(f"idx_{name}", [P, st["n_pos"] * 8], i16, kind="ExternalInput")
        side["mfull"] = nc.dram_tensor(f"m_{name}", [P, st["n_ev"] * P], f8, kind="ExternalInput")
        side["r8"] = nc.dram_tensor(f"r8_{name}", [P, T], f32, kind="ExternalInput")
        side["xres"] = nc.dram_tensor(f"xres_{name}", [P, T * D], bf, kind="ExternalInput")
        side["w"] = nc.dram_tensor(f"w_{name}", [P, 2 * D], f8, kind="ExternalInput")
        side["out"] = nc.dram_tensor(f"out_{name}", [P, T * D], bf, kind="ExternalOutput")
        sides.append(side)

    ident_hbm = nc.dram_tensor("ident", [P, P], bf, kind="ExternalInput")

    with tile.TileContext(nc) as tc, ExitStack() as ctx:
        const = ctx.enter_context(tc.tile_pool(name="const", bufs=1))
        gpool = ctx.enter_context(tc.tile_pool(name="gpool", bufs=24))
        mp = ctx.enter_context(tc.tile_pool(name="mp", bufs=4))
        stbp = ctx.enter_context(tc.tile_pool(name="stbp", bufs=6))
        xrp = ctx.enter_context(tc.tile_pool(name="xrp", bufs=5))
        outp = ctx.enter_context(tc.tile_pool(name="outp", bufs=4))
        st_ps = ctx.enter_context(tc.tile_pool(name="st_ps", bufs=4, space="PSUM"))
        op_ps = ctx.enter_context(tc.tile_pool(name="op_ps", bufs=4, space="PSUM"))

        # SWDGE ucode warm-up: the first dma_gather on a cold core pays ~10us
        # of one-time ucode load; a dummy gather with memset idx (no DMA dep)
        # absorbs it while the preamble const loads stream
        dummy_idx = const.tile([P, 8], i16, tag="didx", name="dummy_idx")
        nc.gpsimd.memset(dummy_idx[:], 0)
        dummy_out = const.tile([P, D], f8, tag="dscr", name="dummy_out")
        nc.gpsimd.dma_gather(
            dummy_out[:].rearrange("p (c e) -> p c e", e=D),
            sides[0]["banks"][0][:, :],
            dummy_idx[:, :],
            P, P, D, queue_num=0,
        )
        gq = [1]  # round-robin SWDGE queue; start at 1 (dummy held q0)

        # gather indices first, split into pieces in gather-issue order:
        # piece 0 (2 gathers) on the vector DMA queue for fastest
        # availability, the rest on the scalar queue (both idle early and
        # independent of the sync queue carrying the other consts)
        for side in sides:
            st, n = side["st"], side["name"]
            pieces, gathers = st["pieces"], st["gathers"]
            side["idxp"] = []
            side["idxp_cols"] = []
            for p in range(len(pieces) - 1):
                g0, g1 = pieces[p], pieces[p + 1]
                c0 = gathers[g0][4] * 8
                c1 = (gathers[g1][4] if g1 < len(gathers) else st["n_pos"]) * 8
                tile_p = const.tile([P, c1 - c0], i16, tag=f"idx_{n}_{p}", name=f"idx_res_{n}_{p}")
                side["idxp"].append(tile_p)
                side["idxp_cols"].append((c0, c1))
        for side in sides:
            st, n = side["st"], side["name"]
            npg = len(st["gathers"])
            B = st["B"]
            tc0 = st["gathers"][npg - B][4] * 8
            tc1 = st["n_pos"] * 8
            side["tail_idx"] = const.tile([P, tc1 - tc0], i16, tag=f"tidx_{n}", name=f"tail_idx_{n}")
            side["tail_c0"] = tc0
            nc.scalar.dma_start(side["tail_idx"][:], side["idx"][:, tc0:tc1])
        for side in sides:
            c0, c1 = side["idxp_cols"][0]
            nc.scalar.dma_start(side["idxp"][0][:], side["idx"][:, c0:c1])
        for p in range(1, 4):
            for side in sides:
                c0, c1 = side["idxp_cols"][p]
                nc.scalar.dma_start(side["idxp"][p][:], side["idx"][:, c0:c1])

        ident_res = const.tile([P, P], bf, tag="ident", name="ident_res")
        nc.sync.dma_start(ident_res[:], ident_hbm[:])
        for side in sides:
            st, n = side["st"], side["name"]
            side["r8_res"] = const.tile([P, st["T"]], f32, tag=f"r8_{n}", name=f"r8_res_{n}")
            nc.sync.dma_start(side["r8_res"][:], side["r8"][:])
            side["wdr"] = const.tile([P, 2 * D], f8, tag=f"wdr_{n}", name=f"wdr_{n}")
            nc.sync.dma_start(side["wdr"][:], side["w"][:, :])

        # gathers are emitted lazily in consumption order (interleaved
        # segments, below); the sliding gather-tile pool provides flow control
        for side in sides:
            side["gtiles"] = {}
            side["g_issued"] = 0
            side["skip"] = set()
        for side in sides:
            st, n = side["st"], side["name"]
            npg = len(st["gathers"])
            B = st["B"]
            for gi in range(npg - B, npg):
                b, s, lo, nch, iofs = st["gathers"][gi]
                gt = const.tile([P, nch * D], f8, tag=f"gtail_{n}_{gi}", name=f"gtail_{n}_{gi}")
                side["gtiles"][(b, s)] = gt
                side["skip"].add(gi)
                out3 = gt[:].rearrange("p (c e) -> p c e", e=D)
                nc.gpsimd.dma_gather(
                    out3,
                    side["banks"][b][:, :],
                    side["tail_idx"][:, (iofs * 8 - side["tail_c0"]) : (iofs * 8 - side["tail_c0"]) + nch * 8],
                    nch * P,
                    nch * P,
                    D,
                    queue_num=gq[0] % 4,
                )
                gq[0] += 1

        def emit_gathers(side, upto_chunk):
            """Issue this side's gathers whose split starts before upto_chunk."""
            st, n = side["st"], side["name"]
            pieces, gathers, scum = st["pieces"], st["gathers"], st["scum"]
            while side["g_issued"] < len(gathers):
                gi = side["g_issued"]
                b, s, lo, nch, iofs = gathers[gi]
                if scum[s] >= upto_chunk:
                    break
                if gi in side["skip"]:
                    side["g_issued"] += 1
                    continue
                p = 0
                while pieces[p + 1] <= gi:
                    p += 1
                pc0 = gathers[pieces[p]][4]
                gt = gpool.tile([P, MAXCH * D], f8, tag="gbuf", name=f"gbuf_{n}")
                side["gtiles"][(b, s)] = gt
                out3 = gt[:, 0 : nch * D].rearrange("p (c e) -> p c e", e=D)
                nc.gpsimd.dma_gather(
                    out3,
                    side["banks"][b][:, :],
                    side["idxp"][p][:, (iofs - pc0) * 8 : (iofs - pc0 + nch) * 8],
                    nch * P,
                    nch * P,
                    D,
                    queue_num=gq[0] % 4,
                )
                gq[0] += 1
                side["g_issued"] += 1

        # interleaved segment schedule: alternate sides so DMA-heavy (u) and
        # compute-heavy (g) phases average out instead of running back-to-back
        sched = []
        iu = ig = 0
        su, sg = sides[0]["st"]["segs"], sides[1]["st"]["segs"]
        while iu < len(su) or ig < len(sg):
            if iu < len(su):
                sched.append((sides[0], su[iu]))
                iu += 1
            if ig < len(sg):
                sched.append((sides[1], sg[ig]))
                ig += 1

        def cpos(st, t):
            """chunk position within a bank of tile t's first chunk."""
            if t >= st["T"]:
                return st["n_pos"] // st["B"]
            return st["gpos_of"][(t, 0, 0)] - st["bank_base"][0]

        def emit_mbuild(side, tlo, thi):
            """one-hot selection matrices for a segment: u built on DVE
            (is_equal, spare vector capacity), g host-prebuilt via the scalar
            DMA queue -- splits the load between DVE and the DMA engines"""
            st, n = side["st"], side["name"]
            B, nch_tile, ev_of = st["B"], st["nch_tile"], st["ev_of"]
            ev0s = ev_of[(tlo, 0, 0)]
            ev1s = ev_of[(thi - 1, B - 1, nch_tile[thi - 1] - 1)] + 1
            nevs = ev1s - ev0s
            Mt = mp.tile([P, nevs * P], f8, tag="m", name=f"m_{n}")
            nc.scalar.dma_start(Mt[:], side["mfull"][:, ev0s * P : ev1s * P])
            return Mt, ev0s

        def emit_xr(side, tlo, thi):
            xr = xrp.tile([P, (thi - tlo) * D], bf, tag="xr", name=f"xr_{side['name']}")
            nc.scalar.dma_start(xr[:], side["xres"][:, tlo * D : thi * D])
            return xr

        # software-pipelined prefetch: M-builds and xr loads run two sched
        # steps (one same-side segment) ahead so segment starts never wait on
        # the ~5us DVE is_equal or the xres DMA
        pre = {}
        for i in range(min(3, len(sched))):
            s_i, (a, b) = sched[i]
            pre[i] = (emit_mbuild(s_i, a, b), emit_xr(s_i, a, b))

        for si, (side, (tlo, thi)) in enumerate(sched):
            st, n = side["st"], side["name"]
            T, B = st["T"], st["B"]
            nch_tile, ev_of, gpos_of = st["nch_tile"], st["ev_of"], st["gpos_of"]
            r8_res = side["r8_res"]
            bank_base = st["bank_base"]
            scum = st["scum"]
            gtiles = side["gtiles"]

            emit_gathers(side, cpos(st, thi))
            if si + 3 < len(sched):
                s_n, (a, b) = sched[si + 3]
                pre[si + 3] = (emit_mbuild(s_n, a, b), emit_xr(s_n, a, b))
            if True:
                ntile = thi - tlo
                (Mt, ev0s), xr = pre.pop(si)
                og = outp.tile([P, ntile * D], bf, tag="og", name=f"og_{n}")

                def do_stage1(t):
                    """stage-1 scatter matmuls + PSUM->SBUF copy; returns stb"""
                    Ct = nch_tile[t] * B
                    stp = st_ps.tile([P, D], f32, tag="st")
                    for h in range(2):  # feature halves: sequential PSUM groups
                        k = 0
                        for b in range(B):
                            for j in range(nch_tile[t]):
                                lp = gpos_of[(t, b, j)] - bank_base[b]
                                kM = ev_of[(t, b, j)] - ev0s
                                sp = bisect.bisect_right(scum, lp) - 1
                                gt = gtiles[(b, sp)]
                                lc = lp - scum[sp]
                                X = gt[:, lc * D + h * P : lc * D + (h + 1) * P]
                                nc.tensor.matmul(
                                    stp[:, h * P : (h + 1) * P], lhsT=X,
                                    rhs=Mt[:, kM * P : (kM + 1) * P],
                                    start=(k == 0), stop=(k == Ct - 1),
                                )
                                k += 1
                    stb = stbp.tile([P, D], f8, tag="stb", name=f"stb_{n}")
                    if t % 2 == 1:
                        nc.vector.tensor_copy(stb[:], stp[:])
                    else:
                        nc.scalar.copy(stb[:], stp[:])
                    return stb

                def do_stage2(t, stb, opre_ap):
                    """stage-2 out_linear (one fp8 DoubleRow matmul: both
                    feature halves contract in a single double-pumped pass)
                    + PSUM-folded residual"""
                    ti_ = t - tlo
                    nc.tensor.matmul(
                        opre_ap,
                        lhsT=stb[:].rearrange("p (h j) -> p h j", j=P),
                        rhs=side["wdr"][:].rearrange("p (h f) -> p h f", f=D),
                        start=True, stop=False,
                        perf_mode=mybir.MatmulPerfMode.DoubleRow,
                    )
                    nc.tensor.matmul(
                        opre_ap, lhsT=ident_res[:],
                        rhs=xr[:, ti_ * D : (ti_ + 1) * D],
                        start=False, stop=True,
                    )

                for t0 in range(tlo, thi, 2):
                    npair = min(2, thi - t0)
                    W = npair * D
                    ti = t0 - tlo
                    opre = op_ps.tile([P, W], f32, tag="opre")
                    # both tiles' stage-1 first: t1's scatter matmuls keep the
                    # PE busy while t0's PSUM->SBUF copy completes
                    stbs = [do_stage1(t0 + q) for q in range(npair)]
                    for q in range(npair):
                        do_stage2(t0 + q, stbs[q], opre[:, q * D : (q + 1) * D])
                    for q in range(npair):
                        tq = t0 + q - tlo
                        if (t0 + q) % 2 == 0:
                            nc.scalar.activation(
                                og[:, tq * D : (tq + 1) * D],
                                opre[:, q * D : (q + 1) * D],
                                mybir.ActivationFunctionType.Relu,
                                scale=r8_res[:, t0 + q : t0 + q + 1],
                            )
                        else:
                            nc.vector.tensor_scalar(
                                out=og[:, tq * D : (tq + 1) * D],
                                in0=opre[:, q * D : (q + 1) * D],
                                scalar1=r8_res[:, t0 + q : t0 + q + 1],
                                scalar2=0.0,
                                op0=mybir.AluOpType.mult,
                                op1=mybir.AluOpType.max,
                            )


                nc.sync.dma_start(side["out"][:, tlo * D : thi * D], og[:])

    nc.compile()
    return nc


_NC_CACHE = {}


def _cfg_key(cfg):
    return (
        cfg["ncores"], cfg["n_user"], cfg["n_game"],
        tuple(sorted(cfg["u"].items())), tuple(sorted(cfg["g"].items())),
    )


def _get_nc(cfg, sts):
    key = _cfg_key(cfg)
    if key not in _NC_CACHE:
        _NC_CACHE[key] = _build(cfg, sts)
    return _NC_CACHE[key]


# ------------------------------------------------------------------- driver

def _prepare(inputs, cfg):
    ncores = cfg["ncores"]
    n_user, n_game = cfg["n_user"], cfg["n_game"]
    uslice, gslice = n_user // ncores, n_game // ncores

    Wb_u, bb_u, bo_u = _fold_weights(
        inputs["Wv_game"], inputs["bv_game"], inputs["Wm_rev"], inputs["bm_rev"],
        inputs["Wout_user"], inputs["bout_user"],
    )
    Wb_g, bb_g, bo_g = _fold_weights(
        inputs["Wv_user"], inputs["bv_user"], inputs["Wm_played"], inputs["bm_played"],
        inputs["Wout_game"], inputs["bout_game"],
    )

    x_user = np.ascontiguousarray(np.float32(inputs["x_user"]))
    x_game = np.ascontiguousarray(np.float32(inputs["x_game"]))
    xu_f8 = x_user.astype(F8)
    xg_f8 = x_game.astype(F8)

    ep_s = np.asarray(inputs["ei_played_src"]).astype(np.int64)
    ep_d = np.asarray(inputs["ei_played_dst"]).astype(np.int64)
    er_s = np.asarray(inputs["ei_rev_src"]).astype(np.int64)
    er_d = np.asarray(inputs["ei_rev_dst"]).astype(np.int64)

    while True:
        sts = _structures(cfg)
        try:
            in_maps = []
            packs = []
            for k in range(ncores):
                sel_u = (er_d >= k * uslice) & (er_d < (k + 1) * uslice)
                pu = _pack_side(
                    sts["u"], er_d[sel_u] - k * uslice, er_s[sel_u],
                    n_game // cfg["u"]["nbanks"], uslice,
                    x_user[k * uslice : (k + 1) * uslice],
                    np.float32(Wb_u), bb_u, bo_u,
                )
                sel_g = (ep_d >= k * gslice) & (ep_d < (k + 1) * gslice)
                pg = _pack_side(
                    sts["g"], ep_d[sel_g] - k * gslice, ep_s[sel_g],
                    n_user // cfg["g"]["nbanks"], gslice,
                    x_game[k * gslice : (k + 1) * gslice],
                    np.float32(Wb_g), bb_g, bo_g,
                )
                packs.append((pu, pg))
                im = dict(
                    ident=np.eye(P, dtype=np.float32).astype(BF16),
                    idx_u=pu["idx"], m_u=pu["mfull"], r8_u=pu["r8pm"], xres_u=pu["xres"],
                    w_u=Wb_u,
                    idx_g=pg["idx"], m_g=pg["mfull"], r8_g=pg["r8pm"], xres_g=pg["xres"],
                    w_g=Wb_g,
                )
                ubank = n_game // cfg["u"]["nbanks"]
                for b in range(cfg["u"]["nbanks"]):
                    im[f"xg{b}"] = xg_f8[b * ubank : (b + 1) * ubank]
                gbank = n_user // cfg["g"]["nbanks"]
                for b in range(cfg["g"]["nbanks"]):
                    im[f"xu{b}"] = xu_f8[b * gbank : (b + 1) * gbank]
                in_maps.append(im)
            break
        except PackError:
            # escalate fat-tile budget (changes structure => recompile)
            cfg = dict(cfg, u=dict(cfg["u"]), g=dict(cfg["g"]))
            cfg["u"]["fat"] += 2
            cfg["u"]["T"] += 1
            cfg["g"]["fat"] += 2
            cfg["g"]["T"] += 1

    return cfg, sts, in_maps, packs


def _run(inputs, cfg=None, trace=False, **run_kwargs):
    cfg = cfg or CFG_FULL
    cfg, sts, in_maps, packs = _prepare(inputs, cfg)
    ncores = cfg["ncores"]
    uslice, gslice = cfg["n_user"] // ncores, cfg["n_game"] // ncores

    nc = _get_nc(cfg, sts)
    res = run_bass_kernel_spmd(nc, in_maps, list(range(ncores)), trace=trace, **run_kwargs)

    def unpack(a, pack, T, nrows):
        a3 = np.asarray(a, dtype=np.float32).reshape(P, T, D)
        return a3[pack["slot_of"], pack["tile_of"], :]

    out_user = np.concatenate(
        [unpack(res.results[k]["out_u"], packs[k][0], sts["u"]["T"], uslice) for k in range(ncores)],
        axis=0,
    )
    out_game = np.concatenate(
        [unpack(res.results[k]["out_g"], packs[k][1], sts["g"]["T"], gslice) for k in range(ncores)],
        axis=0,
    )
    full = np.concatenate([out_user, out_game], axis=0).astype(np.float32)
    return full, res


def kernel(**inputs) -> np.ndarray:
    out, _ = _run(inputs)
    return out

